# revision 32
# baseline (speedup 1.0000x reference)
"""HSTU block-sparse attention (cmp + slc branches) on 8 Trainium2 cores.

Sharding: the 32 (batch, head) pairs are split 4-per-core (core c gets
b = c // 2, heads 4*(c % 2) .. 4*(c % 2)+3). Each core runs the full
per-(b,h) pipeline fused in one Bass/Tile module.

v2 design (instruction-overhead aware):
 - queries processed in two 512-wide supertiles; all heavy matmuls
   stream 512 (or 512-128*j causal-trimmed) columns so the ~200ns fixed
   per-matmul cost is amortized ~4x better than 128-wide tiles.
 - QK and the top-16 block-selection bias are fused into ONE matmul via
   contraction-augmentation: lhsT = [kT; e32] (96 rows), rhs = [q; selb].
 - intra-tile causal bias and the compressed-branch causal bias are
   added on the (otherwise idle) Vector engine straight into PSUM.
 - all scalar-engine activations are Silu/Tanh (sigmoid computed as
   0.5+0.5*tanh(x/2)), which share one activation table -> no 1.3us
   ACT_TABLE_LOADs; psum->sbuf copies moved to the Vector engine.
 - k_cmp / v_cmp block means are single Vector-engine windowed reduces
   (from transposed layouts) instead of 16 PE matmuls.
 - per-query gates are broadcast across partitions with a tiny
   ones-outer-product matmul; gating + branch combine run on Vector.
 - output is produced transposed [64, N] per (b,h) and untransposed on
   the host, which keeps every DMA a clean 2D copy.
"""

import sys

sys.path.insert(0, "/opt/trn_rl_repo")

import numpy as np
import ml_dtypes

B, N, H, D = 4, 1024, 8, 64
BLOCK_SIZE = 32
NB = N // BLOCK_SIZE          # 32 blocks
PAIRS = 4                     # (b,h) pairs per core
NCORES = 8
SCALE = D ** -0.5
MINVAL = -1.0e30
BIGRAW = 1.0e6                # additive mask bias (pre-scale); silu saturates to 0

_CACHE = {}


def _build_statics():
    if "statics" in _CACHE:
        return _CACHE["statics"]
    bf = ml_dtypes.bfloat16
    ident = np.eye(128, dtype=np.float32)
    i_q = np.arange(128)
    # db01[key j, q i] = 1 if i >= j else 0 (intra-tile token causal, post-silu)
    dbias = np.where(i_q[None, :] >= i_q[:, None], 1.0, 0.0).astype(ml_dtypes.bfloat16)
    blk = np.arange(NB)
    qblk = np.arange(N) // BLOCK_SIZE
    # cc01[blk, q] = 1 if blk <= qblk else 0  (compressed-branch causal, post-silu)
    ccT = np.where(blk[:, None] <= qblk[None, :], 1.0, 0.0).astype(ml_dtypes.bfloat16)
    # selcaus[i, t', blk] = +1e30 if blk <= qblk(512 + 128 t' + i) else MINVAL
    qb2 = qblk[512:].reshape(4, 128)
    selcaus = np.where(blk[None, None, :] <= qb2.T[:, :, None],
                       1.0e30, MINVAL).astype(np.float32)
    # e32[j, key] = 1 if key // 32 == j (block expansion over the key axis)
    e32 = (np.arange(N)[None, :] // BLOCK_SIZE == blk[:, None]).astype(np.float32)
    statics = {
        "ident": ident, "dbias": dbias, "ccT": ccT, "selcaus": selcaus,
        "e32": e32,
    }
    _CACHE["statics"] = statics
    return statics


def _build_nc():
    if "nc" in _CACHE:
        return _CACHE["nc"]
    import concourse.bacc as bacc
    import concourse.mybir as mybir
    from concourse.tile import TileContext

    F32 = mybir.dt.float32
    BF16 = mybir.dt.bfloat16
    AF = mybir.ActivationFunctionType
    OP = mybir.AluOpType

    nc = bacc.Bacc("TRN2", target_bir_lowering=False, debug=False,
                   num_devices=NCORES)

    d_kaug = nc.dram_tensor("kaug", [PAIRS, 96, N], BF16, kind="ExternalInput")
    d_qT = nc.dram_tensor("qT", [PAIRS, 64, N], BF16, kind="ExternalInput")
    d_pqT = nc.dram_tensor("pqT", [PAIRS, 64, 512], F32, kind="ExternalInput")
    d_pkT = nc.dram_tensor("pkT", [PAIRS, 64, N], F32, kind="ExternalInput")
    d_vT = nc.dram_tensor("vT", [PAIRS, 64, N], BF16, kind="ExternalInput")
    d_vn = nc.dram_tensor("vn", [PAIRS, N, 64], BF16, kind="ExternalInput")
    d_gwp = nc.dram_tensor("gwp", [PAIRS, 64, 64], BF16, kind="ExternalInput")
    d_id = nc.dram_tensor("ident", [128, 128], F32, kind="ExternalInput")
    d_db = nc.dram_tensor("dbias", [128, 128], BF16, kind="ExternalInput")
    d_cc = nc.dram_tensor("ccT", [NB, N], BF16, kind="ExternalInput")
    d_sc = nc.dram_tensor("selcaus", [128, 4, NB], F32, kind="ExternalInput")
    d_gscr = nc.dram_tensor("gscr", [PAIRS, 2, N], BF16, kind="Internal")
    d_out = nc.dram_tensor("outT", [PAIRS, 64, N], BF16, kind="ExternalOutput")

    with TileContext(nc) as tc:
        with tc.tile_pool(name="sb_c", bufs=1) as sb_c, \
             tc.tile_pool(name="sb_io", bufs=3) as sb_io, \
             tc.tile_pool(name="sb_w", bufs=2) as sb_w, \
             tc.tile_pool(name="ps_p", bufs=3, space="PSUM") as ps_p, \
             tc.tile_pool(name="ps_oo", bufs=3, space="PSUM") as ps_oo, \
             tc.tile_pool(name="ps_sm", bufs=2, space="PSUM") as ps_sm:

            t_id = sb_c.tile([128, 128], F32, tag="t_id")
            nc.sync.dma_start(t_id[:], d_id[:])
            t_db = sb_c.tile([128, 128], BF16, tag="t_db")
            nc.sync.dma_start(t_db[:], d_db[:])
            t_cc = sb_c.tile([NB, N], BF16, tag="t_cc")
            nc.sync.dma_start(t_cc[:], d_cc[:])
            t_sc = sb_c.tile([128, 4, NB], F32, tag="t_sc")
            nc.sync.dma_start(t_sc[:], d_sc[:])

            io = [None] * PAIRS      # per-pair input tiles
            pre = [None] * PAIRS     # per-pair prepass tiles

            def emit_dma(p):
                # reduce/selection inputs first: the DVE reduces and sel
                # matmuls are the longest dependency chain off the DMA
                t_pk = sb_io.tile([64, N], F32, tag="t_pk")
                nc.sync.dma_start(t_pk[:], d_pkT[p])
                t_vT = sb_io.tile([64, N], BF16, tag="t_vT")
                nc.sync.dma_start(t_vT[:], d_vT[p])
                t_pq = sb_io.tile([64, 512], F32, tag="t_pq")
                nc.sync.dma_start(t_pq[:], d_pqT[p])
                t_kcbg = sb_w.tile([64, 64], BF16, tag="t_kcbg")
                nc.sync.dma_start(t_kcbg[:], d_gwp[p])
                t_kaug = sb_io.tile([96, N], BF16, tag="t_kaug")
                nc.sync.dma_start(t_kaug[:], d_kaug[p])
                t_q = sb_io.tile([64, N], BF16, tag="t_q")
                nc.sync.dma_start(t_q[:], d_qT[p])
                t_v = sb_io.tile([128, N // 128, 64], BF16, tag="t_v")
                nc.sync.dma_start(t_v[:], d_vn[p].rearrange("(i q) d -> q i d", q=128))
                io[p] = (t_kaug, t_q, t_pq, t_pk, t_vT, t_v, t_kcbg)

            def emit_pre_mm(p):
                """reduces, augmented-cmp lhsT build, sel scoring + top-16."""
                t_kaug, t_q, t_pq, t_pk, t_vT, t_v, t_kcbg = io[p]
                kcf = sb_w.tile([64, NB], F32, tag="kcf")
                nc.vector.tensor_reduce(
                    kcf[:], t_pk[:].rearrange("p (b t) -> p b t", t=BLOCK_SIZE),
                    mybir.AxisListType.X, OP.add)
                nc.vector.tensor_scalar_mul(t_kcbg[:, 0:32], kcf[:],
                                            1.0 / BLOCK_SIZE)
                vcm = sb_w.tile([64, NB], F32, tag="vcm")
                nc.vector.tensor_reduce(
                    vcm[:], t_vT[:].rearrange("p (b t) -> p b t", t=BLOCK_SIZE),
                    mybir.AxisListType.X, OP.add)
                bqs = []
                for tp in range(4):
                    qs = t_pq[:, 128 * tp:128 * (tp + 1)]
                    p_sel = ps_sm.tile([128, NB], F32, tag="misc")
                    nc.tensor.matmul(p_sel[:], lhsT=qs, rhs=kcf[:],
                                     start=True, stop=True)
                    sm = sb_w.tile([128, NB], F32, tag=f"sm{tp}")
                    nc.vector.tensor_tensor(sm[:], p_sel[:], t_sc[:, tp, :], OP.min)
                    mx = sb_w.tile([128, 8], F32, tag="mx")
                    nc.vector.max(mx[:], sm[:])
                    rep = sb_w.tile([128, NB], F32, tag="rep")
                    nc.vector.match_replace(rep[:], mx[:], sm[:], MINVAL)
                    mx2 = sb_w.tile([128, 8], F32, tag="mx2")
                    nc.vector.max(mx2[:], rep[:])
                    rep2 = sb_w.tile([128, NB], F32, tag="rep2")
                    nc.vector.match_replace(rep2[:], mx2[:], rep[:], MINVAL)
                    dif = sb_w.tile([128, NB], F32, tag="dif")
                    nc.vector.tensor_sub(dif[:], sm[:], rep2[:])
                    nc.vector.tensor_scalar_min(dif[:], dif[:], 1.0)
                    bq = sb_w.tile([128, NB], F32, tag=f"bq{tp}")
                    nc.vector.tensor_scalar(bq[:], dif[:], 1.0, BIGRAW,
                                            OP.subtract, OP.mult)
                    bqs.append(bq)
                pre[p] = (vcm, bqs)

            def emit_pre_fix(p):
                """vcb transpose + selb transposes into the augmented rhs."""
                t_q = io[p][1]
                vcm, bqs = pre[p]
                p_vtp = ps_sm.tile([NB, 64], F32, tag="misc")
                nc.tensor.transpose(p_vtp[:], vcm[:], t_id[0:64, 0:64])
                vcb = sb_w.tile([NB, 64], BF16, tag="vcb")
                nc.vector.tensor_scalar_mul(vcb[:], p_vtp[:], 1.0 / BLOCK_SIZE)
                aq = sb_w.tile([96, 512], BF16, tag="aq")
                nc.vector.tensor_copy(aq[0:64, :], t_q[:, 512:1024])
                for tp in range(4):
                    p_bt = ps_sm.tile([NB, 128], F32, tag="misc")
                    nc.tensor.transpose(p_bt[:], bqs[tp][:], t_id[:])
                    nc.vector.tensor_copy(
                        aq[64:96, 128 * tp:128 * (tp + 1)], p_bt[:])
                pre[p] = pre[p] + (vcb, aq)

            def emit_main_s(p, s, last=False):
                t_kaug, t_q, t_pq, t_pk, t_vT, t_v, t_kcbg = io[p]
                vcb, aq = pre[p][2], pre[p][3]
                ktn = 4 * (s + 1)
                q0 = 512 * s

                # cmp scores + gate scores in one augmented matmul
                t_oo = ps_oo.tile([128, 512], F32, tag="oo")
                p_cg = ps_sm.tile([64, 512], F32, tag="misc")
                nc.tensor.matmul(p_cg[:], lhsT=t_kcbg[:],
                                 rhs=t_q[:, q0:q0 + 512],
                                 start=True, stop=True)
                pcb = sb_w.tile([NB, 512], BF16, tag="pcb")
                nc.scalar.activation(pcb[:], p_cg[0:32, :], AF.Silu, scale=SCALE)
                nc.vector.tensor_mul(pcb[:], pcb[:], t_cc[:, q0:q0 + 512])
                tnh = sb_w.tile([64, 512], F32, tag="tnh")
                nc.scalar.activation(tnh[32:64, :], p_cg[32:64, :], AF.Tanh,
                                     scale=0.5)
                tg = sb_w.tile([64, 512], BF16, tag="tg")
                nc.vector.tensor_scalar(tg[32:64, :], tnh[32:64, :],
                                        0.5, 0.5, OP.mult, OP.add)
                # gate rows -> DRAM scratch -> partition-broadcast reads
                nc.sync.dma_start(d_gscr[p, 0, q0:q0 + 512], tg[32:33, :])
                nc.sync.dma_start(d_gscr[p, 1, q0:q0 + 512], tg[63:64, :])
                gB = sb_w.tile([128, 512], BF16, tag="gBs", bufs=4)
                nc.sync.dma_start(
                    gB[0:64, :],
                    d_gscr[p, 1:2, q0:q0 + 512].broadcast_to([64, 512]))
                nc.sync.dma_start(
                    gB[64:128, :],
                    d_gscr[p, 0:1, q0:q0 + 512].broadcast_to([64, 512]))

                # selected branch over key tiles
                for kt in range(ktn):
                    co = 128 * max(0, kt - 4 * s)
                    ncols = 512 - co
                    p_s = ps_p.tile([128, 512], F32, tag="P")
                    if s == 0:
                        nc.tensor.matmul(
                            p_s[:, 0:ncols],
                            lhsT=t_kaug[0:64, 128 * kt:128 * (kt + 1)],
                            rhs=t_q[:, q0 + co:q0 + 512],
                            start=True, stop=True)
                    else:
                        nc.tensor.matmul(
                            p_s[:, 0:ncols],
                            lhsT=t_kaug[:, 128 * kt:128 * (kt + 1)],
                            rhs=aq[:, co:512],
                            start=True, stop=True)
                    pb = sb_w.tile([128, 512], BF16, tag="pb", bufs=4)
                    nc.scalar.activation(pb[:, 0:ncols], p_s[:, 0:ncols],
                                         AF.Silu, scale=SCALE)
                    if kt >= 4 * s:
                        nc.vector.tensor_mul(pb[:, 0:128], pb[:, 0:128],
                                             t_db[:])
                    nc.tensor.matmul(t_oo[0:64, co:512],
                                     lhsT=t_v[:, kt, :], rhs=pb[:, 0:ncols],
                                     start=(kt == 0), stop=(kt == ktn - 1))

                # compressed-branch output
                nc.tensor.matmul(t_oo[64:128, :], lhsT=vcb[:], rhs=pcb[:],
                                 start=True, stop=True)

                # combine on gpsimd from bf16 sbuf staging
                ooS = sb_w.tile([128, 512], BF16, tag="ooS")
                nc.scalar.copy(ooS[:], t_oo[:])
                o1 = sb_w.tile([64, 512], BF16, tag="o1")
                o2 = sb_w.tile([64, 512], BF16, tag="o2")
                eng = nc.vector if last else nc.gpsimd
                eng.tensor_tensor(o1[:], ooS[64:128, :], gB[64:128, :], OP.mult)
                eng.tensor_tensor(o2[:], ooS[0:64, :], gB[0:64, :], OP.mult)
                eng.tensor_add(o2[:], o2[:], o1[:])
                nc.sync.dma_start(d_out[p, :, q0:q0 + 512], o2[:])

            # software-pipelined emission: prepass runs one pair ahead
            emit_dma(0)
            emit_pre_mm(0)
            emit_pre_fix(0)
            for p in range(PAIRS):
                if p + 1 < PAIRS:
                    emit_dma(p + 1)
                emit_main_s(p, 0)
                if p + 1 < PAIRS:
                    emit_pre_mm(p + 1)
                emit_main_s(p, 1, last=(p == PAIRS - 1))
                if p + 1 < PAIRS:
                    emit_pre_fix(p + 1)

    nc.compile()
    _CACHE["nc"] = nc
    return nc


def _get_runner():
    """Persistent jitted 8-core runner (mirrors run_bass_via_pjrt's
    multi-core branch but caches the jit so repeat calls skip recompiles)."""
    if "runner" in _CACHE:
        return _CACHE["runner"]
    import jax
    import numpy as _np
    from jax.experimental.shard_map import shard_map
    from jax.sharding import Mesh, PartitionSpec
    import concourse.mybir as mybir
    from concourse.bass2jax import (_bass_exec_p, install_neuronx_cc_hook,
                                    partition_id_tensor)

    nc = _build_nc()
    install_neuronx_cc_hook()

    partition_name = (nc.partition_id_tensor.name
                      if nc.partition_id_tensor else None)
    in_names, out_names, out_avals, zero_shapes = [], [], [], []
    for alloc in nc.m.functions[0].allocations:
        if not isinstance(alloc, mybir.MemoryLocationSet):
            continue
        name = alloc.memorylocations[0].name
        if alloc.kind == "ExternalInput":
            if name != partition_name:
                in_names.append(name)
        elif alloc.kind == "ExternalOutput":
            shape = tuple(alloc.tensor_shape)
            dtype = mybir.dt.np(alloc.dtype)
            out_names.append(name)
            out_avals.append(jax.core.ShapedArray(shape, dtype))
            zero_shapes.append((shape, dtype))
    n_params = len(in_names)
    all_names = in_names + out_names
    if partition_name is not None:
        all_names = all_names + [partition_name]

    def _body(*args):
        operands = list(args)
        if partition_name is not None:
            operands.append(partition_id_tensor())
        outs = _bass_exec_p.bind(
            *operands,
            out_avals=tuple(out_avals),
            in_names=tuple(all_names),
            out_names=tuple(out_names),
            lowering_input_output_aliases=(),
            sim_require_finite=True,
            sim_require_nnan=True,
            nc=nc,
        )
        return tuple(outs)

    devices = jax.devices()[:NCORES]
    mesh = Mesh(_np.asarray(devices), ("core",))
    n_outs = len(out_names)
    sharded = jax.jit(
        shard_map(_body, mesh=mesh,
                  in_specs=(PartitionSpec("core"),) * (n_params + n_outs),
                  out_specs=(PartitionSpec("core"),) * n_outs,
                  check_rep=False),
        donate_argnums=tuple(range(n_params, n_params + n_outs)),
        keep_unused=True,
    )

    def run(in_maps):
        concat_in = [
            np.concatenate([in_maps[c][name] for c in range(NCORES)], axis=0)
            for name in in_names
        ]
        concat_zeros = [np.zeros((NCORES * s[0], *s[1:]), dt)
                        for s, dt in zero_shapes]
        out_arrs = sharded(*concat_in, *concat_zeros)
        return [
            {name: np.asarray(out_arrs[i]).reshape(NCORES, *out_avals[i].shape)[c]
             for i, name in enumerate(out_names)}
            for c in range(NCORES)
        ]

    _CACHE["runner"] = run
    return run


def _prepare_in_maps(jagged_q, jagged_k, jagged_v, padded_q, padded_k,
                     padded_v, x_offsets, gate_w, gather_idx):
    bf = ml_dtypes.bfloat16
    st = _build_statics()
    gidx = np.asarray(gather_idx).astype(np.int64)

    # the jagged tensors scattered back to dense are exactly the padded
    # tensors (padding rows are zeroed in both), so only padded_* is needed.
    pq = np.asarray(padded_q, np.float32)
    pk = np.asarray(padded_k, np.float32)
    pv = np.asarray(padded_v, np.float32)
    gw = np.asarray(gate_w, np.float32)
    e32b = st["e32"].astype(bf)

    in_maps = []
    for c in range(NCORES):
        b = c // 2
        hs = [4 * (c % 2) + j for j in range(PAIRS)]
        kaug = np.stack([
            np.concatenate([pk[b, :, h, :].T.astype(bf), e32b], axis=0)
            for h in hs])
        qT = np.stack([pq[b, :, h, :].T for h in hs]).astype(bf)
        pqT = np.stack([pq[b, 512:, h, :].T for h in hs]).astype(np.float32)
        pkT = np.stack([pk[b, :, h, :].T for h in hs]).astype(np.float32)
        vT = np.stack([pv[b, :, h, :].T for h in hs]).astype(bf)
        vn = np.stack([pv[b, :, h, :] for h in hs]).astype(bf)
        gwc = np.zeros((PAIRS, 64, 64), np.float32)
        for j, h in enumerate(hs):
            gwc[j, :, 32] = gw[h, :, 0]
            gwc[j, :, 63] = gw[h, :, 1]
        gwc = gwc.astype(bf)
        in_maps.append({
            "kaug": np.ascontiguousarray(kaug),
            "qT": np.ascontiguousarray(qT),
            "pqT": np.ascontiguousarray(pqT),
            "pkT": np.ascontiguousarray(pkT),
            "vT": np.ascontiguousarray(vT),
            "vn": np.ascontiguousarray(vn),
            "gwp": np.ascontiguousarray(gwc),
            "ident": st["ident"], "dbias": st["dbias"], "ccT": st["ccT"],
            "selcaus": st["selcaus"],
        })
    return in_maps, gidx


def kernel(jagged_q, jagged_k, jagged_v, jagged_u, padded_q, padded_k,
           padded_v, x_offsets, gate_w, padding_mask, gather_idx):
    in_maps, gidx = _prepare_in_maps(jagged_q, jagged_k, jagged_v, padded_q,
                                     padded_k, padded_v, x_offsets, gate_w,
                                     gather_idx)
    run = _get_runner()
    results = run(in_maps)
    o_dense = np.zeros((B, N, H, D), np.float32)
    for c in range(NCORES):
        b = c // 2
        hs = [4 * (c % 2) + j for j in range(PAIRS)]
        out = results[c]["outT"].astype(np.float32)
        for p, h in enumerate(hs):
            o_dense[b, :, h, :] = out[p].T
    return o_dense.reshape(B * N, H, D)[gidx]
